# revision 5
# baseline (speedup 1.0000x reference)
"""Multi-head attention (B=4, N=2048, DIM=512, H=8, DH=64) on 8 TRN2 cores.

Sharding: core c handles batch b = c//2 and head group g = c%2 (4 heads).
Each core computes the qkv projection for its 4 heads, full attention, and a
partial output projection (its heads' rows of w_out, plus b_out/2). Host
sums the two partials per batch.

v3 design (transposed-PV rework of v2; HW median 190us vs v2's 282us
measured interleaved in one session — the device itself drifts 10-20%
between windows, so only same-session comparisons are meaningful):
  - PV runs with P^T as the stationary operand and V as the moving operand:
    out psO2[h] is [128 queries, 65] per 128-query chunk (64 dh cols + a
    denominator column from V's ones column). Each PV matmul streams only 65
    rows instead of 512, halving PE time on the PV half of attention. The
    pend-queue lag (2 units) keeps the in-order PE queue from parking on ACT;
    PV quads additionally interleave between S^T pairs (ILV) so their
    stationary loads hide under the 512-row S^T streams.
  - Work is split into 8 passes of (head pair p, query quarter qh): psO2 is
    2 heads x [128, 4*65] f32 = 2 PSUM banks, psS keeps 2 double-buffered
    [128, 1024] slots (4 banks), and a 2-slot "background" pool (2 banks)
    takes every other PSUM user (q/k/V projections, transposes, psY) so the
    S^T slots that pace ACT are never stolen.
  - Normalization is per-partition: gather the strided denominator columns,
    reciprocal_approx_fast, 4 tensor_scalar multiplies into bf16 O_sb.
    No broadcast matmuls, no partition-base copy dance.
  - O_sb [q, dh] is PE-transposed (bf16 identity) into [dh, q] ot tiles for
    the output projection; b_out rides as a K=1 ones x bh matmul into psY,
    and y DMAs straight from PSUM (no SBUF staging).
  - ACT (ScalarE) does ONLY the softmax exp (128 x [128,1024] activations,
    ~133us): the kernel is ACT-throughput-bound; all PE work (S^T, PV,
    projections, transposes, out-proj) fits in PE slack woven after each
    unit's S^T pair.

Precision: bf16 x/w/P/V/O + fp32r q/k (same as v2; rel err ~6.7e-3, gate 2e-2).
"""

import os
from contextlib import ExitStack

import numpy as np

ABLATE = set()      # ablation hooks retained for dev A/B, disabled here
TAILMODE = "fast"
BIASMODE = "ap"
ILV = True          # interleave PV quads between S^T pairs (hides ldweights)

import concourse.bass as bass
import concourse.tile as tile
from concourse import bacc, masks, mybir

N = 2048          # sequence length
NQ = N // 4       # 512: queries per pass (quarter)
DIM = 512         # model dim
DH = 64           # head dim
HC = 4            # heads per core
HD = HC * DH      # 256: per-core head width
KC = DIM // 128   # 4 contraction chunks for the projections
NT = N // 128     # 16 key tiles
MP = NT // 2      # 8 key-tile pairs per pass
FB = 512          # matmul free-dim block
LAG = 4           # pend entries outstanding before PV flush (2 units)
SCALE = DH ** -0.5
EXP_BIAS = -3.8   # softmax shift; cancels in normalization

f32 = mybir.dt.float32
f32r = mybir.dt.float32r
bf16 = mybir.dt.bfloat16
EXP = mybir.ActivationFunctionType.Exp
COPY = mybir.ActivationFunctionType.Copy


def emit_attention(ctx: ExitStack, tc: tile.TileContext, xT, wq, wk, wv, wo, bh, y):
    nc = tc.nc

    consts = ctx.enter_context(tc.tile_pool(name="consts", bufs=1))
    inputs = ctx.enter_context(tc.tile_pool(name="inputs", bufs=1))
    acts = ctx.enter_context(tc.tile_pool(name="acts", bufs=1))
    pt_pool = ctx.enter_context(tc.tile_pool(name="pt", bufs=7))
    osb_pool = ctx.enter_context(tc.tile_pool(name="osb", bufs=1))
    ot_pool = ctx.enter_context(tc.tile_pool(name="ot", bufs=1))
    dn_pool = ctx.enter_context(tc.tile_pool(name="dn", bufs=1))
    y_pool = ctx.enter_context(tc.tile_pool(name="ys", bufs=4))

    # PSUM (8 banks): 2x2-bank S^T slots + 2 banks of psO2 accumulators +
    # 2 rotating 1-bank background slots (projections, transposes, psY).
    pS = ctx.enter_context(tc.tile_pool(name="pS", bufs=2, space="PSUM"))
    pO2 = ctx.enter_context(tc.tile_pool(name="pO2", bufs=1, space="PSUM"))
    pBG = ctx.enter_context(tc.tile_pool(name="pBG", bufs=2, space="PSUM"))

    def bg_tile(shape, dtype=f32):
        return pBG.tile(shape, dtype, tag="bg", name="ps_bg")

    ones_bf = consts.tile([1, 128], bf16)
    nc.vector.memset(ones_bf[:], 1.0)
    ebias = consts.tile([128, 1], f32)
    nc.vector.memset(ebias[:], EXP_BIAS)
    ident = consts.tile([128, 128], bf16)
    masks.make_identity(nc, ident[:])

    # ---- inputs, DMA'd straight into bf16 SBUF tiles (batched APs) ----
    xT_s = inputs.tile([128, KC * N], bf16)       # chunk c at cols [c*N, ...)
    wq_s = inputs.tile([128, KC * HD], bf16)
    wk_s = inputs.tile([128, KC * HD], bf16)
    wv_s = inputs.tile([128, KC * HD], bf16)
    # head pair concat: even head at partitions 0-63, odd at 64-127
    wo_s = inputs.tile([128, 2 * DIM], bf16)      # pair g rows at cols g*DIM
    bh_s = inputs.tile([1, DIM], bf16)

    # split input DMA issue across the two HWDGE queues (SP and ACT — ACT is
    # idle until the first exp): SP carries wk + xT, ACT carries wq/wv/wo/bh
    wk_v = wk_s[:].rearrange("p (c h) -> p c h", c=KC)
    wq_v = wq_s[:].rearrange("p (c h) -> p c h", c=KC)
    wv_v = wv_s[:].rearrange("p (c h) -> p c h", c=KC)
    nc.sync.dma_start(wk_v, wk[:].rearrange("(c p) h -> p c h", p=128))
    nc.scalar.dma_start(wq_v, wq[:].rearrange("(c p) h -> p c h", p=128))
    nc.scalar.dma_start(wv_v, wv[:].rearrange("(c p) h -> p c h", p=128))
    xT_v = xT_s[:].rearrange("p (c n) -> p c n", c=KC)
    xT_d = xT[:].rearrange("(c p) n -> p c n", p=128)
    for n in range(4):
        nc.sync.dma_start(xT_v[:, :, n * FB:(n + 1) * FB],
                          xT_d[:, :, n * FB:(n + 1) * FB])
    for h in range(HC):
        r0 = (h % 2) * DH
        nc.scalar.dma_start(wo_s[r0:r0 + DH, (h // 2) * DIM:(h // 2 + 1) * DIM],
                            wo[h * DH:(h + 1) * DH, :])
    nc.scalar.dma_start(bh_s[:], bh[:, :])

    # ---- V (bf16) with a ones column: [128, key tile jt, head h, 65] ----
    V_s = acts.tile([128, NT * HC * (DH + 1)], bf16)
    V_sr = V_s[:].rearrange("p (j h d) -> p j h d", j=NT, h=HC)
    nc.vector.memset(V_sr[:, :, :, DH:DH + 1], 1.0)

    def emit_v_proj(jt):
        ps = bg_tile([128, HD])
        for c in range(KC):
            nc.tensor.matmul(
                ps[:],
                xT_s[:, c * N + jt * 128: c * N + (jt + 1) * 128],
                wv_s[:, c * HD:(c + 1) * HD],
                start=(c == 0), stop=(c == KC - 1),
            )
        nc.vector.tensor_copy(
            V_sr[:, jt, :, 0:DH], ps[:].rearrange("p (h d) -> p h d", d=DH))

    # ---- q/k projections, transposed: pair p head parity at partition 0/64 ----
    qT_s = acts.tile([128, 2 * N], f32r)
    kT_s = acts.tile([128, 2 * N], f32r)

    def emit_qk_group(p, w_s, o_s, n):
        ps = bg_tile([128, FB])
        for c in range(KC):
            nc.tensor.matmul(
                ps[:],
                w_s[:, c * HD + p * 128: c * HD + (p + 1) * 128],
                xT_s[:, c * N + n * FB: c * N + (n + 1) * FB],
                start=(c == 0), stop=(c == KC - 1),
            )
        nc.vector.tensor_copy(o_s[:, p * N + n * FB: p * N + (n + 1) * FB], ps[:])

    # ---- attention ----
    # per-(pair, quarter) [dh x 2 heads, 512 queries] tiles: even head rows
    # 0-63, odd 64-127 — lhsT for the output projection
    ot_tiles = {(p, qh): ot_pool.tile([128, NQ], bf16, tag=f"ot{p}_{qh}",
                                      name=f"ot{p}_{qh}")
                for p in range(2) for qh in range(4)}
    # normalized [query, dh] staging per head, rewritten every pass
    osb_tiles = {h: osb_pool.tile([128, HD], bf16, tag=f"osb{h}",
                                  name=f"osb{h}")
                 for h in range(HC)}
    dn_tiles = {h: dn_pool.tile([128, 4], f32, tag=f"dn{h}", name=f"dn{h}")
                for h in range(HC)}

    def attention(p, qh, extra_work=None, fast=False):
        """One pass: heads (2p, 2p+1) x queries [qh*512, (qh+1)*512).

        extra_work: {m: [fns]} emitted AFTER unit m's S^T pair (the in-order
        PE queue makes any weave placed before the consumer correct; placing
        it after the S^T keeps ACT fed)."""
        work = extra_work or {}
        heads = (2 * p, 2 * p + 1)
        q0 = p * N + qh * NQ
        psO2 = {h: pO2.tile([128, 4 * (DH + 1)], f32, tag=f"o{h % 2}",
                            name="psO2") for h in heads}
        if "pv" in ABLATE:      # keep psO2 written so later reads are legal
            for h in heads:
                nc.vector.memset(psO2[h][:], 1.0)
        pt0 = None
        if "exp" in ABLATE:     # one const P tile replaces the exp outputs
            pt0 = pt_pool.tile([128, 2 * NQ], bf16, tag="pt0", name="pt0")
            nc.vector.memset(pt0[:], 0.001)
        pend = []
        pvq = []   # PV quarter-flushes (4 matmuls each) awaiting emission

        def emit_pv_quad():
            h, m, t, pt = pvq.pop(0)
            jt = 2 * m + t
            for qc in range(4):
                # one accumulation group per head bank: start zeroes the
                # whole 2KB zero region, so chunks qc>0 accumulate onto
                # pending-zero bytes
                nc.tensor.matmul(
                    psO2[h][:, qc * (DH + 1):(qc + 1) * (DH + 1)],
                    pt[:, t * NQ + qc * 128: t * NQ + (qc + 1) * 128],
                    V_sr[:, jt, h],
                    start=(jt == 0 and qc == 0),
                    stop=(jt == NT - 1 and qc == 3),
                )

        def flush_pv():
            h, m, pt = pend.pop(0)
            if "pv" in ABLATE:
                return
            for t in range(2):
                pvq.append((h, m, t, pt))
            if not ILV:
                while pvq:
                    emit_pv_quad()

        for m in range(MP):
            # S^T pair FIRST: strictly alternates the two heads' 64-row
            # groups so consecutive matmuls overlap on the PE array. With
            # ILV, pending PV quads slot between the t-pairs so their
            # stationary loads hide under the 512-row S^T streams.
            psSs = [pS.tile([128, 2 * NQ], f32, tag="s", name="ps_s")
                    for _ in heads]
            if "st" not in ABLATE:
                for t in range(2):
                    jt = 2 * m + t
                    for hi, h in enumerate(heads):
                        row0 = hi * DH
                        psSr = psSs[hi][:].rearrange("p (t f) -> p t f", t=2)
                        nc.tensor.matmul(
                            psSr[:, t, :],
                            kT_s[row0:row0 + DH, p * N + jt * 128: p * N + (jt + 1) * 128],
                            qT_s[row0:row0 + DH, q0: q0 + NQ],
                            start=True, stop=True,
                        )
                    if ILV and pvq:
                        emit_pv_quad()
            for hi, h in enumerate(heads):
                if "exp" in ABLATE:
                    pend.append((h, m, pt0))
                    continue
                pt = pt_pool.tile([128, 2 * NQ], bf16, tag="pt", name="pt")
                bias = EXP_BIAS if BIASMODE == "imm" else ebias[:]
                nc.scalar.activation(pt[:], psSs[hi][:], EXP, scale=SCALE,
                                     bias=bias)
                pend.append((h, m, pt))
            for fn_ in work.get(m, ()):
                fn_()
            while len(pend) > LAG:
                flush_pv()
            while len(pvq) > 2:         # keep <=2 quads for next unit's gaps
                emit_pv_quad()
        for fn_ in work.get(MP, ()):    # pre-tail weave (e.g. V14/V15)
            fn_()
        while pend:
            flush_pv()
        while pvq:
            emit_pv_quad()
        # normalization: DVE-only, emitted now; the PE transposes are woven
        # into the next pass via emit_evac. fast=True interleaves chunk-major
        # for the final pass's short-critical-chain tail.
        if fast:
            for h in heads:
                dn = dn_tiles[h]
                nc.vector.tensor_copy(
                    dn[:].rearrange("p (c o) -> p c o", o=1),
                    psO2[h][:].rearrange("p (c x) -> p c x", x=DH + 1)[:, :, DH:DH + 1])
                nc.vector.reciprocal_approx_fast(out=dn[:], in_=dn[:])
            for c in range(4):
                for h in heads:
                    nc.vector.tensor_scalar_mul(
                        osb_tiles[h][:, c * DH:(c + 1) * DH],
                        psO2[h][:, c * (DH + 1): c * (DH + 1) + DH],
                        dn_tiles[h][:, c:c + 1])
        else:
            for h in heads:
                dn = dn_tiles[h]
                nc.vector.tensor_copy(
                    dn[:].rearrange("p (c o) -> p c o", o=1),
                    psO2[h][:].rearrange("p (c x) -> p c x", x=DH + 1)[:, :, DH:DH + 1])
                nc.vector.reciprocal_approx_fast(out=dn[:], in_=dn[:])
                for c in range(4):
                    nc.vector.tensor_scalar_mul(
                        osb_tiles[h][:, c * DH:(c + 1) * DH],
                        psO2[h][:, c * (DH + 1): c * (DH + 1) + DH],
                        dn[:, c:c + 1])

    def emit_evac(p, qh, hi):
        """PE-transpose head (2p+hi)'s normalized [q, dh] into ot rows."""
        h = 2 * p + hi
        psT = bg_tile([DH, NQ], bf16)
        for c in range(4):
            nc.tensor.transpose(psT[:, c * 128:(c + 1) * 128],
                                osb_tiles[h][:, c * DH:(c + 1) * DH],
                                ident[:])
        nc.vector.tensor_copy(ot_tiles[(p, qh)][hi * DH:(hi + 1) * DH, :],
                              psT[:])

    def emit_out_proj(qh, c, act_copy=False):
        psY = bg_tile([128, DIM])
        for g in range(2):
            nc.tensor.matmul(
                psY[:],
                ot_tiles[(g, qh)][:, c * 128:(c + 1) * 128],
                wo_s[:, g * DIM:(g + 1) * DIM],
                start=(g == 0), stop=False,
            )
        nc.tensor.matmul(psY[:], ones_bf[:], bh_s[:], start=False, stop=True)
        ys = y_pool.tile([128, DIM], f32, tag="ys", name="ys")
        if act_copy:    # tail: ACT is idle after the last exp, DVE is not
            nc.scalar.activation(ys[:], psY[:], COPY)
        else:
            nc.vector.tensor_copy(ys[:], psY[:])
        nt = 4 * qh + c
        nc.sync.dma_start(y[nt * 128:(nt + 1) * 128, :], ys[:])

    def emit_fast_tail(p, qh):
        """Chunk-granular evac + out-proj for the final pass. The transposes
        land in a now-idle S^T slot (both heads, all chunks in one tile), the
        per-chunk DVE copies release each out-proj as early as possible, and
        the two bg slots are left free for the psY rotation."""
        psT2 = pS.tile([128, NQ], bf16, tag="s", name="ps_t2")
        # transpose both heads chunk-major into [q-chunk c][head rows]
        for c in range(4):
            for hi in range(2):
                h = 2 * p + hi
                nc.tensor.transpose(
                    psT2[hi * DH:(hi + 1) * DH, c * 128:(c + 1) * 128],
                    osb_tiles[h][:, c * DH:(c + 1) * DH],
                    ident[:])
        for c in range(4):
            nc.vector.tensor_copy(
                ot_tiles[(p, qh)][:, c * 128:(c + 1) * 128],
                psT2[:, c * 128:(c + 1) * 128])
            emit_out_proj(qh, c, act_copy=True)

    # ---- schedule ----
    # upfront: what pass (0,0) needs immediately + the first 4 V tiles
    emit_qk_group(0, wk_s, kT_s, 0)
    emit_qk_group(0, wq_s, qT_s, 0)
    for jt in range(4):
        emit_v_proj(jt)

    def qk(p, w_s, o_s, n):
        return lambda: emit_qk_group(p, w_s, o_s, n)

    def vp(jt):
        return lambda: emit_v_proj(jt)

    def ev(p, qh, hi):
        return lambda: emit_evac(p, qh, hi)

    def op(qh, c):
        return lambda: emit_out_proj(qh, c)

    # pass A (0,0): V tiles just-in-time + rest of pair-0 k blocks + pair-1
    # k/q startup for pass B
    attention(0, 0, {
        0: [vp(4), vp(5)],
        1: [qk(0, wk_s, kT_s, 1)],
        2: [vp(6), vp(7)],
        3: [qk(0, wk_s, kT_s, 2)],
        4: [vp(8), vp(9)],
        5: [qk(0, wk_s, kT_s, 3)],
        6: [vp(10), vp(11)],
        7: [vp(12), vp(13)],
        8: [vp(14), vp(15), qk(1, wk_s, kT_s, 0), qk(1, wq_s, qT_s, 0)],
    })
    # pass B (1,0): pair-0 qh-0 evac + remaining pair-1 k blocks + q lookahead
    attention(1, 0, {
        0: [qk(1, wk_s, kT_s, 1)],
        1: [ev(0, 0, 0)],
        2: [ev(0, 0, 1)],
        3: [qk(1, wk_s, kT_s, 2)],
        4: [qk(1, wk_s, kT_s, 3)],
        5: [qk(0, wq_s, qT_s, 1)],
        6: [qk(1, wq_s, qT_s, 1)],
    })
    # pass C (0,1): pair-1 qh-0 evac + quarter-0 output projection
    attention(0, 1, {
        1: [ev(1, 0, 0)],
        2: [ev(1, 0, 1)],
        3: [op(0, 0)],
        4: [op(0, 1)],
        5: [op(0, 2)],
        6: [op(0, 3)],
    })
    # pass D (1,1)
    attention(1, 1, {
        1: [ev(0, 1, 0)],
        2: [ev(0, 1, 1)],
        3: [qk(0, wq_s, qT_s, 2)],
    })
    # pass E (0,2)
    attention(0, 2, {
        1: [ev(1, 1, 0)],
        2: [ev(1, 1, 1)],
        3: [op(1, 0)],
        4: [op(1, 1)],
        5: [op(1, 2)],
        6: [op(1, 3)],
        7: [qk(1, wq_s, qT_s, 2)],
    })
    # pass F (1,2)
    attention(1, 2, {
        1: [ev(0, 2, 0)],
        2: [ev(0, 2, 1)],
        3: [qk(0, wq_s, qT_s, 3)],
    })
    # pass G (0,3)
    attention(0, 3, {
        1: [ev(1, 2, 0)],
        2: [ev(1, 2, 1)],
        3: [op(2, 0)],
        4: [op(2, 1)],
        5: [op(2, 2)],
        6: [op(2, 3)],
        7: [qk(1, wq_s, qT_s, 3)],
    })
    # pass H (1,3)
    attention(1, 3, {
        1: [ev(0, 3, 0)],
        2: [ev(0, 3, 1)],
    }, fast=(TAILMODE == "fast"))
    # tail: evac pass H + quarter-3 output projection
    if TAILMODE == "fast":
        emit_fast_tail(1, 3)
    else:
        emit_evac(1, 3, 0)
        emit_evac(1, 3, 1)
        for c in range(4):
            emit_out_proj(3, c)


def build_nc(for_hw: bool = True, reps: int = 1, hw_loop: bool = False) -> bass.Bass:
    # Bacc (not raw Bass): its compile pipeline splits multi-wait sync
    # conditions, which the TRN2 ISA caps at one per instruction.
    nc = bacc.Bacc()
    xT = nc.declare_dram_parameter("xT", [DIM, N], bf16, isOutput=False)
    wq = nc.declare_dram_parameter("wq", [DIM, HD], bf16, isOutput=False)
    wk = nc.declare_dram_parameter("wk", [DIM, HD], bf16, isOutput=False)
    wv = nc.declare_dram_parameter("wv", [DIM, HD], bf16, isOutput=False)
    wo = nc.declare_dram_parameter("wo", [HD, DIM], bf16, isOutput=False)
    bh = nc.declare_dram_parameter("bh", [1, DIM], bf16, isOutput=False)
    y = nc.declare_dram_parameter("y", [N, DIM], f32, isOutput=True)
    with tile.TileContext(nc) as tc:
        if hw_loop and reps > 1:
            with tc.For_i(0, reps, 1):
                with ExitStack() as ctx:
                    emit_attention(ctx, tc, xT[:], wq[:], wk[:], wv[:], wo[:], bh[:], y[:])
        else:
            for _ in range(reps):
                with ExitStack() as ctx:
                    emit_attention(ctx, tc, xT[:], wq[:], wk[:], wv[:], wo[:], bh[:], y[:])
    if for_hw:
        nc.finalize()
    else:
        nc.compile()
    return nc


def shard_inputs(x, w_qkv, w_out, b_out) -> list[dict]:
    import ml_dtypes
    bf = ml_dtypes.bfloat16
    x = np.asarray(x, dtype=np.float32)
    w_qkv = np.asarray(w_qkv, dtype=np.float32)
    w_out = np.asarray(w_out, dtype=np.float32)
    b_out = np.asarray(b_out, dtype=np.float32)
    in_maps = []
    for c in range(8):
        b, g = c // 2, c % 2
        in_maps.append({
            "xT": np.ascontiguousarray(x[b].T).astype(bf),
            "wq": np.ascontiguousarray(w_qkv[:, g * HD:(g + 1) * HD]).astype(bf),
            "wk": np.ascontiguousarray(w_qkv[:, DIM + g * HD: DIM + (g + 1) * HD]).astype(bf),
            "wv": np.ascontiguousarray(w_qkv[:, 2 * DIM + g * HD: 2 * DIM + (g + 1) * HD]).astype(bf),
            "wo": np.ascontiguousarray(w_out[g * HD:(g + 1) * HD, :]).astype(bf),
            "bh": (b_out * 0.5)[None, :].astype(bf),
        })
    return in_maps


def run_sharded(x, w_qkv, w_out, b_out, trace=False, **kw):
    from concourse.bass_utils import run_bass_kernel_spmd

    nc = build_nc()
    in_maps = shard_inputs(x, w_qkv, w_out, b_out)
    res = run_bass_kernel_spmd(nc, in_maps, list(range(8)), trace=trace, **kw)
    parts = [res.results[c]["y"] for c in range(8)]
    out = np.stack([parts[2 * b] + parts[2 * b + 1] for b in range(4)])
    return out.astype(np.float32), res


def kernel(x, mask, w_qkv, w_out, b_out):
    out, _ = run_sharded(x, w_qkv, w_out, b_out)
    return out


# revision 9
# speedup vs baseline: 1.0385x; 1.0385x over previous
"""Multi-head attention (B=4, N=2048, DIM=512, H=8, DH=64) on 8 TRN2 cores.

Sharding: core c handles batch b = c//2 and head group g = c%2 (4 heads).
Each core computes the qkv projection for its 4 heads, full attention, and a
partial output projection (its heads' rows of w_out, plus b_out/2). Host
sums the two partials per batch.

v3 design (transposed-PV rework of v2):
  - PV runs with P^T as the stationary operand and V as the moving operand:
    out psO2[h] is [128 queries, 65] per 128-query chunk (64 dh cols + a
    denominator column from V's ones column). Each PV matmul streams only 65
    rows instead of 512, halving PE time on the PV half of attention. The
    pend-queue lag (2 units) keeps the in-order PE queue from parking on ACT.
  - Work is split into 8 passes of (head pair p, query quarter qh): psO2 is
    2 heads x [128, 4*65] f32 = 2 PSUM banks, psS keeps 2 double-buffered
    [128, 1024] slots (4 banks), and a 2-slot "background" pool (2 banks)
    takes every other PSUM user (q/k/V projections, transposes, psY) so the
    S^T slots that pace ACT are never stolen.
  - Normalization is per-partition: gather the strided denominator columns,
    reciprocal_approx_fast, 4 tensor_scalar multiplies into bf16 O_sb.
    No broadcast matmuls, no partition-base copy dance.
  - O_sb [q, dh] is PE-transposed (bf16 identity) into [dh, q] ot tiles for
    the output projection; b_out rides as a K=1 ones x bh matmul into psY,
    and y DMAs straight from PSUM (no SBUF staging).
  - ACT (ScalarE) does ONLY the softmax exp (128 x [128,1024] activations,
    ~133us): the kernel is ACT-throughput-bound; all PE work (S^T, PV,
    projections, transposes, out-proj) fits in PE slack woven after each
    unit's S^T pair.

Precision: bf16 x/w/P/V/O + fp32r q/k (same as v2; rel err ~6.7e-3, gate 2e-2).
"""

import os
from contextlib import ExitStack

import numpy as np

ABLATE = set()      # ablation hooks retained for dev A/B, disabled here
TAILMODE = "fast"
BIASMODE = "ap"
ILV = True          # interleave PV quads between S^T pairs (hides ldweights)

import concourse.bass as bass
import concourse.tile as tile
from concourse import bacc, masks, mybir

N = 2048          # sequence length
NQ = N // 4       # 512: queries per pass (quarter)
DIM = 512         # model dim
DH = 64           # head dim
HC = 4            # heads per core
HD = HC * DH      # 256: per-core head width
KC = DIM // 128   # 4 contraction chunks for the projections
NT = N // 128     # 16 key tiles
MP = NT // 2      # 8 key-tile pairs per pass
FB = 512          # matmul free-dim block
LAG = 4           # pend entries outstanding before PV flush (2 units)
SCALE = DH ** -0.5
EXP_BIAS = -3.8   # softmax shift; cancels in normalization

f32 = mybir.dt.float32
f32r = mybir.dt.float32r
bf16 = mybir.dt.bfloat16
EXP = mybir.ActivationFunctionType.Exp
COPY = mybir.ActivationFunctionType.Copy


def emit_attention(ctx: ExitStack, tc: tile.TileContext, xT, wq, wk, wv, wo, bh, y):
    nc = tc.nc

    consts = ctx.enter_context(tc.tile_pool(name="consts", bufs=1))
    inputs = ctx.enter_context(tc.tile_pool(name="inputs", bufs=1))
    acts = ctx.enter_context(tc.tile_pool(name="acts", bufs=1))
    pt_pool = ctx.enter_context(tc.tile_pool(name="pt", bufs=LAG + 4))
    osb_pool = ctx.enter_context(tc.tile_pool(name="osb", bufs=1))
    ot_pool = ctx.enter_context(tc.tile_pool(name="ot", bufs=1))
    dn_pool = ctx.enter_context(tc.tile_pool(name="dn", bufs=1))
    y_pool = ctx.enter_context(tc.tile_pool(name="ys", bufs=4))

    # PSUM (8 banks): 2x2-bank S^T slots + 2 banks of psO2 accumulators +
    # 2 rotating 1-bank background slots (projections, transposes, psY).
    pS = ctx.enter_context(tc.tile_pool(name="pS", bufs=2, space="PSUM"))
    pO2 = ctx.enter_context(tc.tile_pool(name="pO2", bufs=1, space="PSUM"))
    pBG = ctx.enter_context(tc.tile_pool(name="pBG", bufs=2, space="PSUM"))

    def bg_tile(shape, dtype=f32):
        return pBG.tile(shape, dtype, tag="bg", name="ps_bg")

    ones_bf = consts.tile([1, 128], bf16)
    nc.vector.memset(ones_bf[:], 1.0)
    ebias = consts.tile([128, 1], f32)
    nc.vector.memset(ebias[:], EXP_BIAS)
    ident = consts.tile([128, 128], bf16)
    masks.make_identity(nc, ident[:])

    # ---- inputs, DMA'd straight into bf16 SBUF tiles (batched APs) ----
    xT_s = inputs.tile([128, KC * N], bf16)       # chunk c at cols [c*N, ...)
    wq_s = inputs.tile([128, KC * HD], bf16)
    wk_s = inputs.tile([128, KC * HD], bf16)
    wv_s = inputs.tile([128, KC * HD], bf16)
    # head pair concat: even head at partitions 0-63, odd at 64-127
    wo_s = inputs.tile([128, 2 * DIM], bf16)      # pair g rows at cols g*DIM
    bh_s = inputs.tile([1, DIM], bf16)

    # split input DMA issue across the two HWDGE queues (SP and ACT — ACT is
    # idle until the first exp): SP carries wk + xT, ACT carries wq/wv/wo/bh
    wk_v = wk_s[:].rearrange("p (c h) -> p c h", c=KC)
    wq_v = wq_s[:].rearrange("p (c h) -> p c h", c=KC)
    wv_v = wv_s[:].rearrange("p (c h) -> p c h", c=KC)
    nc.sync.dma_start(wk_v, wk[:].rearrange("(c p) h -> p c h", p=128))
    nc.scalar.dma_start(wq_v, wq[:].rearrange("(c p) h -> p c h", p=128))
    nc.scalar.dma_start(wv_v, wv[:].rearrange("(c p) h -> p c h", p=128))
    xT_v = xT_s[:].rearrange("p (c n) -> p c n", c=KC)
    xT_d = xT[:].rearrange("(c p) n -> p c n", p=128)
    for n in range(4):
        nc.sync.dma_start(xT_v[:, :, n * FB:(n + 1) * FB],
                          xT_d[:, :, n * FB:(n + 1) * FB])
    for h in range(HC):
        r0 = (h % 2) * DH
        nc.scalar.dma_start(wo_s[r0:r0 + DH, (h // 2) * DIM:(h // 2 + 1) * DIM],
                            wo[h * DH:(h + 1) * DH, :])
    nc.scalar.dma_start(bh_s[:], bh[:, :])

    # ---- V (bf16) with a ones column: [128, key tile jt, head h, 65] ----
    V_s = acts.tile([128, NT * HC * (DH + 1)], bf16)
    V_sr = V_s[:].rearrange("p (j h d) -> p j h d", j=NT, h=HC)
    nc.vector.memset(V_sr[:, :, :, DH:DH + 1], 1.0)

    def emit_v_proj(jt):
        ps = bg_tile([128, HD])
        for c in range(KC):
            nc.tensor.matmul(
                ps[:],
                xT_s[:, c * N + jt * 128: c * N + (jt + 1) * 128],
                wv_s[:, c * HD:(c + 1) * HD],
                start=(c == 0), stop=(c == KC - 1),
            )
        nc.vector.tensor_copy(
            V_sr[:, jt, :, 0:DH], ps[:].rearrange("p (h d) -> p h d", d=DH))

    # ---- q/k projections, transposed: pair p head parity at partition 0/64 ----
    qT_s = acts.tile([128, 2 * N], f32r)
    kT_s = acts.tile([128, 2 * N], f32r)

    def emit_qk_group(p, w_s, o_s, n):
        ps = bg_tile([128, FB])
        for c in range(KC):
            nc.tensor.matmul(
                ps[:],
                w_s[:, c * HD + p * 128: c * HD + (p + 1) * 128],
                xT_s[:, c * N + n * FB: c * N + (n + 1) * FB],
                start=(c == 0), stop=(c == KC - 1),
            )
        nc.vector.tensor_copy(o_s[:, p * N + n * FB: p * N + (n + 1) * FB], ps[:])

    # ---- attention ----
    # per-(pair, quarter) [dh x 2 heads, 512 queries] tiles: even head rows
    # 0-63, odd 64-127 — lhsT for the output projection
    ot_tiles = {(p, qh): ot_pool.tile([128, NQ], bf16, tag=f"ot{p}_{qh}",
                                      name=f"ot{p}_{qh}")
                for p in range(2) for qh in range(4)}
    # normalized [query, dh] staging per head, rewritten every pass
    osb_tiles = {h: osb_pool.tile([128, HD], bf16, tag=f"osb{h}",
                                  name=f"osb{h}")
                 for h in range(HC)}
    dn_tiles = {h: dn_pool.tile([128, 4], f32, tag=f"dn{h}", name=f"dn{h}")
                for h in range(HC)}

    def attention(p, qh, extra_work=None, fast=False, defer_tail=False):
        """One pass: heads (2p, 2p+1) x queries [qh*512, (qh+1)*512).

        extra_work: {m: [fns]} emitted AFTER unit m's S^T pair (the in-order
        PE queue makes any weave placed before the consumer correct; placing
        it after the S^T keeps ACT fed). defer_tail=True returns the tail
        (pre-tail weave, remaining PV flushes, norms) as a closure list for
        the caller to weave into the NEXT pass instead of emitting inline —
        shrinks the inter-pass PE chain that would starve ACT."""
        work = extra_work or {}
        heads = (2 * p, 2 * p + 1)
        q0 = p * N + qh * NQ
        psO2 = {h: pO2.tile([128, 4 * (DH + 1)], f32, tag=f"o{h % 2}",
                            name="psO2") for h in heads}
        if "pv" in ABLATE:      # keep psO2 written so later reads are legal
            for h in heads:
                nc.vector.memset(psO2[h][:], 1.0)
        pt0 = None
        if "exp" in ABLATE:     # one const P tile replaces the exp outputs
            pt0 = pt_pool.tile([128, 2 * NQ], bf16, tag="pt0", name="pt0")
            nc.vector.memset(pt0[:], 0.001)
        pend = []
        pvq = []   # PV quarter-flushes (4 matmuls each) awaiting emission

        def emit_pv_quad():
            h, m, t, pt = pvq.pop(0)
            jt = 2 * m + t
            for qc in range(4):
                # one accumulation group per head bank: start zeroes the
                # whole 2KB zero region, so chunks qc>0 accumulate onto
                # pending-zero bytes
                nc.tensor.matmul(
                    psO2[h][:, qc * (DH + 1):(qc + 1) * (DH + 1)],
                    pt[:, t * NQ + qc * 128: t * NQ + (qc + 1) * 128],
                    V_sr[:, jt, h],
                    start=(jt == 0 and qc == 0),
                    stop=(jt == NT - 1 and qc == 3),
                )

        def flush_pv():
            h, m, pt = pend.pop(0)
            if "pv" in ABLATE:
                return
            for t in range(2):
                pvq.append((h, m, t, pt))
            if not ILV:
                while pvq:
                    emit_pv_quad()

        for m in range(MP):
            # S^T pair FIRST: strictly alternates the two heads' 64-row
            # groups so consecutive matmuls overlap on the PE array. With
            # ILV, pending PV quads slot between the t-pairs so their
            # stationary loads hide under the 512-row S^T streams.
            psSs = [pS.tile([128, 2 * NQ], f32, tag="s", name="ps_s")
                    for _ in heads]
            if "st" not in ABLATE:
                for t in range(2):
                    jt = 2 * m + t
                    for hi, h in enumerate(heads):
                        row0 = hi * DH
                        psSr = psSs[hi][:].rearrange("p (t f) -> p t f", t=2)
                        nc.tensor.matmul(
                            psSr[:, t, :],
                            kT_s[row0:row0 + DH, p * N + jt * 128: p * N + (jt + 1) * 128],
                            qT_s[row0:row0 + DH, q0: q0 + NQ],
                            start=True, stop=True,
                        )
                    if ILV and pvq:
                        emit_pv_quad()
            for hi, h in enumerate(heads):
                if "exp" in ABLATE:
                    pend.append((h, m, pt0))
                    continue
                pt = pt_pool.tile([128, 2 * NQ], bf16, tag="pt", name="pt")
                bias = EXP_BIAS if BIASMODE == "imm" else ebias[:]
                nc.scalar.activation(pt[:], psSs[hi][:], EXP, scale=SCALE,
                                     bias=bias)
                pend.append((h, m, pt))
            for fn_ in work.get(m, ()):
                fn_()
            while len(pend) > LAG:
                flush_pv()
            while len(pvq) > 2:         # keep <=2 quads for next unit's gaps
                emit_pv_quad()
        def emit_norm():
            emit_norms(p, qh, heads, psO2, fast)

        if defer_tail:
            items = list(work.get(MP, ()))

            def _one_flush():
                flush_pv()
                while pvq:
                    emit_pv_quad()

            items += [_one_flush] * len(pend)
            items.append(emit_norm)
            return items
        for fn_ in work.get(MP, ()):    # pre-tail weave (e.g. V14/V15)
            fn_()
        while pend:
            flush_pv()
        while pvq:
            emit_pv_quad()
        emit_norm()
        return None

    def emit_norms(p, qh, heads, psO2, fast):
        # normalization: DVE-only; the PE transposes are woven into the next
        # pass via emit_evac. fast=True interleaves chunk-major for the
        # final pass's short-critical-chain tail.
        if fast:
            for h in heads:
                dn = dn_tiles[h]
                nc.vector.tensor_copy(
                    dn[:].rearrange("p (c o) -> p c o", o=1),
                    psO2[h][:].rearrange("p (c x) -> p c x", x=DH + 1)[:, :, DH:DH + 1])
                nc.vector.reciprocal_approx_fast(out=dn[:], in_=dn[:])
            for c in range(4):
                for h in heads:
                    nc.vector.tensor_scalar_mul(
                        osb_tiles[h][:, c * DH:(c + 1) * DH],
                        psO2[h][:, c * (DH + 1): c * (DH + 1) + DH],
                        dn_tiles[h][:, c:c + 1])
        else:
            for h in heads:
                dn = dn_tiles[h]
                nc.vector.tensor_copy(
                    dn[:].rearrange("p (c o) -> p c o", o=1),
                    psO2[h][:].rearrange("p (c x) -> p c x", x=DH + 1)[:, :, DH:DH + 1])
                nc.vector.reciprocal_approx_fast(out=dn[:], in_=dn[:])
                for c in range(4):
                    nc.vector.tensor_scalar_mul(
                        osb_tiles[h][:, c * DH:(c + 1) * DH],
                        psO2[h][:, c * (DH + 1): c * (DH + 1) + DH],
                        dn[:, c:c + 1])

    def emit_evac(p, qh, hi):
        """PE-transpose head (2p+hi)'s normalized [q, dh] into ot rows."""
        h = 2 * p + hi
        psT = bg_tile([DH, NQ], bf16)
        for c in range(4):
            nc.tensor.transpose(psT[:, c * 128:(c + 1) * 128],
                                osb_tiles[h][:, c * DH:(c + 1) * DH],
                                ident[:])
        nc.vector.tensor_copy(ot_tiles[(p, qh)][hi * DH:(hi + 1) * DH, :],
                              psT[:])

    def emit_out_proj(qh, c, act_copy=False):
        psY = bg_tile([128, DIM])
        for g in range(2):
            nc.tensor.matmul(
                psY[:],
                ot_tiles[(g, qh)][:, c * 128:(c + 1) * 128],
                wo_s[:, g * DIM:(g + 1) * DIM],
                start=(g == 0), stop=False,
            )
        nc.tensor.matmul(psY[:], ones_bf[:], bh_s[:], start=False, stop=True)
        ys = y_pool.tile([128, DIM], f32, tag="ys", name="ys")
        if act_copy:    # tail: ACT is idle after the last exp, DVE is not
            nc.scalar.activation(ys[:], psY[:], COPY)
        else:
            nc.vector.tensor_copy(ys[:], psY[:])
        nt = 4 * qh + c
        nc.sync.dma_start(y[nt * 128:(nt + 1) * 128, :], ys[:])

    def emit_fast_tail(p, qh):
        """Chunk-granular evac + out-proj for the final pass. The transposes
        land in a now-idle S^T slot (both heads, all chunks in one tile), the
        per-chunk DVE copies release each out-proj as early as possible, and
        the two bg slots are left free for the psY rotation."""
        psT2 = pS.tile([128, NQ], bf16, tag="s", name="ps_t2")
        # transpose both heads chunk-major into [q-chunk c][head rows]
        for c in range(4):
            for hi in range(2):
                h = 2 * p + hi
                nc.tensor.transpose(
                    psT2[hi * DH:(hi + 1) * DH, c * 128:(c + 1) * 128],
                    osb_tiles[h][:, c * DH:(c + 1) * DH],
                    ident[:])
        for c in range(4):
            nc.vector.tensor_copy(
                ot_tiles[(p, qh)][:, c * 128:(c + 1) * 128],
                psT2[:, c * 128:(c + 1) * 128])
            emit_out_proj(qh, c, act_copy=True)

    # ---- schedule ----
    # upfront: what pass (0,0) needs immediately + the first UPV V tiles
    # (the rest weave into pass A's early units, which still have slack)
    UPV = 4
    emit_qk_group(0, wk_s, kT_s, 0)
    emit_qk_group(0, wq_s, qT_s, 0)
    for jt in range(UPV):
        emit_v_proj(jt)

    def qk(p, w_s, o_s, n):
        return lambda: emit_qk_group(p, w_s, o_s, n)

    def vp(jt):
        return lambda: emit_v_proj(jt)

    def ev(p, qh, hi):
        return lambda: emit_evac(p, qh, hi)

    def op(qh, c):
        return lambda: emit_out_proj(qh, c)

    # pass A (0,0): V tiles just-in-time + rest of pair-0 k blocks + pair-1
    # k/q startup for pass B. V deadlines (LAG=4): V(jt) before the flush at
    # unit jt//2+2; k(0,n) before unit 2n; in-order PE makes placement-before
    # sufficient.
    workA = {m: [] for m in range(9)}
    vq = list(range(UPV, NT))   # V tiles not emitted upfront
    for m in (1, 3, 5):
        workA[m].append(qk(0, wk_s, kT_s, (m + 1) // 2))
    for m in (0, 2, 4, 6, 7):
        for _ in range(2):
            if vq:
                workA[m].append(vp(vq.pop(0)))
    while vq:
        workA[8].append(vp(vq.pop(0)))
    tailA = attention(0, 0, workA, defer_tail=True)
    # boundary: only pass B's hard deps sit between the passes; pass A's
    # tail (pre-tail V tiles, 4 flushes, norms) weaves into B's early units.
    # A's norms must precede B's first own flush (unit 2) — psO2 bufs=1.
    emit_qk_group(1, wk_s, kT_s, 0)
    emit_qk_group(1, wq_s, qT_s, 0)
    # pass B (1,0): pair-0 qh-0 evac + remaining pair-1 k blocks + q lookahead
    attention(1, 0, {
        0: [qk(1, wk_s, kT_s, 1)] + tailA[:2],
        1: tailA[2:],
        2: [ev(0, 0, 0)],
        3: [ev(0, 0, 1), qk(1, wk_s, kT_s, 2)],
        4: [qk(1, wk_s, kT_s, 3)],
        5: [qk(0, wq_s, qT_s, 1)],
        6: [qk(1, wq_s, qT_s, 1)],
    })
    # pass C (0,1): pair-1 qh-0 evac + quarter-0 output projection
    attention(0, 1, {
        1: [ev(1, 0, 0)],
        2: [ev(1, 0, 1)],
        3: [op(0, 0)],
        4: [op(0, 1)],
        5: [op(0, 2)],
        6: [op(0, 3)],
    })
    # pass D (1,1)
    attention(1, 1, {
        1: [ev(0, 1, 0)],
        2: [ev(0, 1, 1)],
        3: [qk(0, wq_s, qT_s, 2)],
    })
    # pass E (0,2)
    attention(0, 2, {
        1: [ev(1, 1, 0)],
        2: [ev(1, 1, 1)],
        3: [op(1, 0)],
        4: [op(1, 1)],
        5: [op(1, 2)],
        6: [op(1, 3)],
        7: [qk(1, wq_s, qT_s, 2)],
    })
    # pass F (1,2)
    attention(1, 2, {
        1: [ev(0, 2, 0)],
        2: [ev(0, 2, 1)],
        3: [qk(0, wq_s, qT_s, 3)],
    })
    # pass G (0,3)
    attention(0, 3, {
        1: [ev(1, 2, 0)],
        2: [ev(1, 2, 1)],
        3: [op(2, 0)],
        4: [op(2, 1)],
        5: [op(2, 2)],
        6: [op(2, 3)],
        7: [qk(1, wq_s, qT_s, 3)],
    })
    # pass H (1,3)
    attention(1, 3, {
        1: [ev(0, 3, 0)],
        2: [ev(0, 3, 1)],
    }, fast=(TAILMODE == "fast"))
    # tail: evac pass H + quarter-3 output projection
    if TAILMODE == "fast":
        emit_fast_tail(1, 3)
    else:
        emit_evac(1, 3, 0)
        emit_evac(1, 3, 1)
        for c in range(4):
            emit_out_proj(3, c)


def build_nc(for_hw: bool = True, reps: int = 1, hw_loop: bool = False) -> bass.Bass:
    # Bacc (not raw Bass): its compile pipeline splits multi-wait sync
    # conditions, which the TRN2 ISA caps at one per instruction.
    nc = bacc.Bacc()
    xT = nc.declare_dram_parameter("xT", [DIM, N], bf16, isOutput=False)
    wq = nc.declare_dram_parameter("wq", [DIM, HD], bf16, isOutput=False)
    wk = nc.declare_dram_parameter("wk", [DIM, HD], bf16, isOutput=False)
    wv = nc.declare_dram_parameter("wv", [DIM, HD], bf16, isOutput=False)
    wo = nc.declare_dram_parameter("wo", [HD, DIM], bf16, isOutput=False)
    bh = nc.declare_dram_parameter("bh", [1, DIM], bf16, isOutput=False)
    y = nc.declare_dram_parameter("y", [N, DIM], f32, isOutput=True)
    with tile.TileContext(nc) as tc:
        if hw_loop and reps > 1:
            with tc.For_i(0, reps, 1):
                with ExitStack() as ctx:
                    emit_attention(ctx, tc, xT[:], wq[:], wk[:], wv[:], wo[:], bh[:], y[:])
        else:
            for _ in range(reps):
                with ExitStack() as ctx:
                    emit_attention(ctx, tc, xT[:], wq[:], wk[:], wv[:], wo[:], bh[:], y[:])
    if for_hw:
        nc.finalize()
    else:
        nc.compile()
    return nc


def shard_inputs(x, w_qkv, w_out, b_out) -> list[dict]:
    import ml_dtypes
    bf = ml_dtypes.bfloat16
    x = np.asarray(x, dtype=np.float32)
    w_qkv = np.asarray(w_qkv, dtype=np.float32)
    w_out = np.asarray(w_out, dtype=np.float32)
    b_out = np.asarray(b_out, dtype=np.float32)
    in_maps = []
    for c in range(8):
        b, g = c // 2, c % 2
        in_maps.append({
            "xT": np.ascontiguousarray(x[b].T).astype(bf),
            "wq": np.ascontiguousarray(w_qkv[:, g * HD:(g + 1) * HD]).astype(bf),
            "wk": np.ascontiguousarray(w_qkv[:, DIM + g * HD: DIM + (g + 1) * HD]).astype(bf),
            "wv": np.ascontiguousarray(w_qkv[:, 2 * DIM + g * HD: 2 * DIM + (g + 1) * HD]).astype(bf),
            "wo": np.ascontiguousarray(w_out[g * HD:(g + 1) * HD, :]).astype(bf),
            "bh": (b_out * 0.5)[None, :].astype(bf),
        })
    return in_maps


def run_sharded(x, w_qkv, w_out, b_out, trace=False, **kw):
    from concourse.bass_utils import run_bass_kernel_spmd

    nc = build_nc()
    in_maps = shard_inputs(x, w_qkv, w_out, b_out)
    res = run_bass_kernel_spmd(nc, in_maps, list(range(8)), trace=trace, **kw)
    parts = [res.results[c]["y"] for c in range(8)]
    out = np.stack([parts[2 * b] + parts[2 * b + 1] for b in range(4)])
    return out.astype(np.float32), res


def kernel(x, mask, w_qkv, w_out, b_out):
    out, _ = run_sharded(x, w_qkv, w_out, b_out)
    return out
